# revision 27
# baseline (speedup 1.0000x reference)
"""Trainium2 Bass kernel for CALayer (squeeze-excitation channel attention).

Reference computation (per batch sample b):
    y  = mean(x[b], spatial)              # [C]
    y1 = leaky_relu(w1 @ y + b1, 0.2)     # [16]
    y2 = sigmoid(w2 @ y1 + b2)            # [C]
    out[b] = x[b] * y2[:, None, None]

Sharding: data-parallel over batch — 8 samples, 8 NeuronCores, one sample per
core, weights replicated, no cross-core communication.

Per-core plan (memory-bound, x[b] = 16 MiB fits in SBUF):
  - x[b] viewed as [256, 16384] lives in SBUF as two [128, 16384] channel
    halves; loaded in column chunks (half0 via the SP HWDGE ring, half1 via
    the ACT HWDGE ring) so pooling overlaps the loads.
  - Pooling: DVE reduce_sum per chunk for half0; ACT Copy-with-accum_out for
    half1 (both engines in parallel, hidden under DMA).
  - Gate: PE matmuls against pre-transposed weights (transposed on host),
    LeakyReLU decomposed as max(t, 0.2*t) on DVE, Sigmoid+bias on ACT
    (table set pre-warmed at kernel start so the ~2.7us load hides under DMA).
  - Scale: per-partition broadcast multiply by the gate — DVE tensor_scalar
    for half0, ACT Copy-with-scale for half1, in place, chunked; stores
    issued per chunk on the GPSIMD SWDGE ring (engine otherwise idle).

HBM traffic per core: 16 MiB in + 16 MiB out (the roofline for this op).
"""

from contextlib import ExitStack

import numpy as np

import concourse.bacc as bacc
import concourse.bass as bass
import concourse.mybir as mybir
import concourse.tile as tile
from concourse.bass_utils import run_bass_kernel_spmd

F32 = mybir.dt.float32
AF = mybir.ActivationFunctionType
ALU = mybir.AluOpType
AX = mybir.AxisListType

B, C, H, W = 8, 256, 128, 128
S = H * W          # 16384 spatial elements
CS = 16            # squeezed channels
NEG_SLOPE = 0.2
N_CORES = 8
P = 128            # SBUF partitions


def _plans(s):
    """(load_widths, scale_widths) per half. Loads taper down so the last
    pooled chunk is small (shrinks the gate bubble); scales taper up so the
    first store fires quickly after the gate."""
    if s == 16384:
        lw = [4096, 4096, 4096, 2048, 1024, 1024]
        return lw, lw[::-1]
    n = max(1, s // 512)
    return [s // n] * n, [s // n] * n


def _body(tc, x, w1t, b1, w2t, b2, out, s, load_w=None, scale_w=None):
    """Emit the per-core kernel. APs: x/out [C, s], w1t [C, CS], b1 [CS, 1],
    w2t [CS, C], b2 [C, 1]."""
    nc = tc.nc
    if load_w is None:
        load_w, scale_w = _plans(s)
    assert sum(load_w) == s and sum(scale_w) == s
    xr = x.rearrange("(h p) s -> h p s", p=P)       # [2, 128, s]
    outr = out.rearrange("(h p) s -> h p s", p=P)

    with ExitStack() as ctx:
        data = ctx.enter_context(tc.tile_pool(name="data", bufs=1))
        small = ctx.enter_context(tc.tile_pool(name="small", bufs=1))
        psum = ctx.enter_context(tc.tile_pool(name="psum", bufs=1, space="PSUM"))

        # Persistent SBUF halves of x (channel c on partition, spatial on free)
        xt0 = data.tile([P, s], F32)
        xt1 = data.tile([P, s], F32)

        # Constants. w1t packed [p, h, CS] so one DMA loads both halves;
        # b2 packed [p, h].
        w1_raw = small.tile([P, 2, CS], F32)
        w2_raw = small.tile([CS, C], F32)
        w1_sb = small.tile([P, 2, CS], F32)
        w2_sb = small.tile([CS, C], F32)
        b1_sb = small.tile([CS, 1], F32)
        b2_sb = small.tile([P, 2], F32)
        nc.gpsimd.dma_start(out=w1_raw, in_=w1t.rearrange("(h p) c -> p h c", p=P))
        nc.gpsimd.dma_start(out=w2_raw, in_=w2t)
        nc.gpsimd.dma_start(out=b1_sb, in_=b1)
        nc.gpsimd.dma_start(out=b2_sb, in_=b2.rearrange("(h p) o -> p (h o)", p=P))
        # Stage the matmul weights through DVE: PE LDWEIGHTS can encode only
        # ONE sync wait, so every matmul must depend on a single semaphore
        # (DVE's) — never on a DMA-lane sem + DVE at once.
        nc.vector.tensor_copy(w1_sb, w1_raw)
        nc.vector.tensor_copy(w2_sb, w2_raw)

        # Three independent DMA paths: SP HWDGE ring, ACT HWDGE ring, GPSIMD
        # SWDGE ring. Round-robin the big transfers across them.
        rings = [nc.sync, nc.scalar, nc.gpsimd]

        # Phase A: load x + pool. Emit ALL load triggers before the first
        # ACT activation so the ACT table load doesn't delay the ACT-ring
        # DMAs. part*[p, j] hold per-chunk partial sums.
        nld = len(load_w)
        part0 = small.tile([P, nld], F32)
        offs = []
        o = 0
        for w in load_w:
            offs.append(o)
            o += w
        # Loads round-robin across all three rings (2 HWDGE + SWDGE).
        ring_i = 0
        for j, w in enumerate(load_w):
            sl = slice(offs[j], offs[j] + w)
            for h, xt in ((0, xt0), (1, xt1)):
                rings[ring_i % 3].dma_start(out=xt[:, sl], in_=xr[h, :, sl])
                ring_i += 1

        # Warm the ACT sigmoid table set while the DMAs stream (must be ACT's
        # first ACTIVATE so Sigmoid/Copy share one table-set load).
        warm = small.tile([1, 1], F32)
        nc.vector.memset(warm, 0.0)
        nc.scalar.activation(out=warm, in_=warm, func=AF.Sigmoid)

        # half0 on DVE: read-only reduce. (In-place tensor_scalar with
        # accum_out is 2x faster on DVE but its SBUF write traffic
        # throttles the concurrent load DMAs ~430 -> ~270 GB/s.)
        part1 = small.tile([P, nld], F32)
        scr_pool = ctx.enter_context(tc.tile_pool(name="scratch", bufs=2))
        for j, w in enumerate(load_w):
            sl = slice(offs[j], offs[j] + w)
            nc.vector.reduce_sum(
                out=part0[:, j : j + 1], in_=xt0[:, sl], axis=AX.X
            )
            # half1 on ACT: Copy to scratch with accumulate.
            scr = scr_pool.tile([P, max(load_w)], F32, tag="scr")
            nc.scalar.activation(
                out=scr[:, :w],
                in_=xt1[:, sl],
                func=AF.Copy,
                bias=0.0,
                scale=1.0,
                accum_out=part1[:, j : j + 1],
            )

        # Gate: combine partials, two tiny GEMVs on PE, LeakyReLU, Sigmoid.
        sums0 = small.tile([P, 1], F32)
        sums1 = small.tile([P, 1], F32)
        nc.vector.reduce_sum(out=sums0, in_=part0, axis=AX.X)
        nc.vector.reduce_sum(out=sums1, in_=part1, axis=AX.X)

        py1 = psum.tile([CS, 1], F32)
        nc.tensor.matmul(py1, w1_sb[:, 0, :], sums0, start=True, stop=False)
        nc.tensor.matmul(py1, w1_sb[:, 1, :], sums1, start=False, stop=True)

        # t = py1/s + b1 ; y1 = max(t, 0.2*t)  (== leaky_relu(t))
        t = small.tile([CS, 1], F32)
        u = small.tile([CS, 1], F32)
        y1_sb = small.tile([CS, 1], F32)
        nc.vector.tensor_scalar(t, py1, 1.0 / s, b1_sb, ALU.mult, ALU.add)
        nc.vector.tensor_scalar_mul(u, t, NEG_SLOPE)
        nc.vector.tensor_max(y1_sb, t, u)

        py20 = psum.tile([P, 1], F32)
        py21 = psum.tile([P, 1], F32)
        nc.tensor.matmul(py20, w2_sb[:, 0:P], y1_sb, start=True, stop=True)
        nc.tensor.matmul(py21, w2_sb[:, P : 2 * P], y1_sb, start=True, stop=True)

        y2_sb = small.tile([P, 2], F32)
        nc.scalar.activation(
            out=y2_sb[:, 0:1], in_=py20, func=AF.Sigmoid, bias=b2_sb[:, 0:1], scale=1.0
        )
        nc.scalar.activation(
            out=y2_sb[:, 1:2], in_=py21, func=AF.Sigmoid, bias=b2_sb[:, 1:2], scale=1.0
        )

        # Phase B: scale x by the gate in place and store, chunked so DMA-out
        # overlaps the multiplies. DVE takes half0, ACT takes half1; stores
        # round-robin over the three DMA rings.
        o = 0
        for w in scale_w:
            sl = slice(o, o + w)
            o += w
            nc.vector.tensor_scalar_mul(
                out=xt0[:, sl], in0=xt0[:, sl], scalar1=y2_sb[:, 0:1]
            )
            nc.scalar.activation(
                out=xt1[:, sl], in_=xt1[:, sl], func=AF.Copy, bias=0.0,
                scale=y2_sb[:, 1:2],
            )
            rings[ring_i % 3].dma_start(out=outr[0, :, sl], in_=xt0[:, sl])
            ring_i += 1
            rings[ring_i % 3].dma_start(out=outr[1, :, sl], in_=xt1[:, sl])
            ring_i += 1


def build_calayer_bass(s=S, trn_type="TRN2"):
    # Bacc (not raw Bass): its compile() pipeline splits multi-wait sync_info
    # into event semaphores — TRN2 instructions encode at most one wait.
    nc = bacc.Bacc(trn_type=trn_type)
    x = nc.dram_tensor("x", [C, s], F32, kind="ExternalInput")
    w1t = nc.dram_tensor("w1t", [C, CS], F32, kind="ExternalInput")
    b1 = nc.dram_tensor("b1", [CS, 1], F32, kind="ExternalInput")
    w2t = nc.dram_tensor("w2t", [CS, C], F32, kind="ExternalInput")
    b2 = nc.dram_tensor("b2", [C, 1], F32, kind="ExternalInput")
    out = nc.dram_tensor("out", [C, s], F32, kind="ExternalOutput")
    with tile.TileContext(nc) as tc:
        _body(
            tc, x[:, :], w1t[:, :], b1[:, :], w2t[:, :], b2[:, :], out[:, :], s,
        )
    nc.finalize()  # Bacc.finalize runs compile(): wait-splitting, reg alloc
    return nc


_NC_CACHE = None
RUN_KWARGS = {}      # test harness may inject trace=True etc.
LAST_RESULT = None   # BassKernelResults of the most recent run


def _get_nc():
    global _NC_CACHE
    if _NC_CACHE is None:
        _NC_CACHE = build_calayer_bass()
    return _NC_CACHE


def kernel(x, w1, b1, w2, b2):
    global LAST_RESULT
    x = np.asarray(x, dtype=np.float32)
    xf = np.ascontiguousarray(x.reshape(B, C, S))
    w1t_h = np.ascontiguousarray(np.asarray(w1, dtype=np.float32).T)  # [C, CS]
    w2t_h = np.ascontiguousarray(np.asarray(w2, dtype=np.float32).T)  # [CS, C]
    b1_h = np.ascontiguousarray(np.asarray(b1, dtype=np.float32).reshape(CS, 1))
    b2_h = np.ascontiguousarray(np.asarray(b2, dtype=np.float32).reshape(C, 1))

    in_maps = [
        {"x": xf[b], "w1t": w1t_h, "b1": b1_h, "w2t": w2t_h, "b2": b2_h}
        for b in range(B)
    ]
    res = run_bass_kernel_spmd(
        _get_nc(), in_maps, core_ids=list(range(N_CORES)), **RUN_KWARGS
    )
    LAST_RESULT = res
    out = np.stack([res.results[b]["out"] for b in range(B)], axis=0)
    return out.reshape(B, C, H, W)


# revision 29
# speedup vs baseline: 1.0894x; 1.0894x over previous
"""Trainium2 Bass kernel for CALayer (squeeze-excitation channel attention).

Reference computation (per batch sample b):
    y  = mean(x[b], spatial)              # [C]
    y1 = leaky_relu(w1 @ y + b1, 0.2)     # [16]
    y2 = sigmoid(w2 @ y1 + b2)            # [C]
    out[b] = x[b] * y2[:, None, None]

Sharding: data-parallel over batch — 8 samples, 8 NeuronCores, one sample per
core, weights replicated, no cross-core communication.

Per-core plan (memory-bound, x[b] = 16 MiB fits in SBUF):
  - x[b] viewed as [256, 16384] lives in SBUF as two [128, 16384] channel
    halves; loaded in column chunks (half0 via the SP HWDGE ring, half1 via
    the ACT HWDGE ring) so pooling overlaps the loads.
  - Pooling: DVE reduce_sum per chunk for half0; ACT Copy-with-accum_out for
    half1 (both engines in parallel, hidden under DMA).
  - Gate: PE matmuls against pre-transposed weights (transposed on host),
    LeakyReLU decomposed as max(t, 0.2*t) on DVE, Sigmoid+bias on ACT
    (table set pre-warmed at kernel start so the ~2.7us load hides under DMA).
  - Scale: per-partition broadcast multiply by the gate — DVE tensor_scalar
    for half0, ACT Copy-with-scale for half1, in place, chunked; stores
    issued per chunk on the GPSIMD SWDGE ring (engine otherwise idle).

HBM traffic per core: 16 MiB in + 16 MiB out (the roofline for this op).
"""

from contextlib import ExitStack

import numpy as np

import concourse.bacc as bacc
import concourse.bass as bass
import concourse.mybir as mybir
import concourse.tile as tile
from concourse.bass_utils import run_bass_kernel_spmd

F32 = mybir.dt.float32
AF = mybir.ActivationFunctionType
ALU = mybir.AluOpType
AX = mybir.AxisListType

B, C, H, W = 8, 256, 128, 128
S = H * W          # 16384 spatial elements
CS = 16            # squeezed channels
NEG_SLOPE = 0.2
N_CORES = 8
P = 128            # SBUF partitions


def _plans(s):
    """(load_widths, scale_widths) per half. Loads taper down so the last
    pooled chunk is small (shrinks the gate bubble); scales taper up so the
    first store fires quickly after the gate."""
    if s == 16384:
        lw = [4096, 4096, 4096, 2048, 1024, 1024]
        return lw, lw[::-1]
    n = max(1, s // 512)
    return [s // n] * n, [s // n] * n


def _body(tc, x, w1t, b1, w2t, b2, out, s, load_w=None, scale_w=None):
    """Emit the per-core kernel. APs: x/out [C, s], w1t [C, CS], b1 [CS, 1],
    w2t [CS, C], b2 [C, 1]."""
    nc = tc.nc
    if load_w is None:
        load_w, scale_w = _plans(s)
    assert sum(load_w) == s and sum(scale_w) == s
    xr = x.rearrange("(h p) s -> h p s", p=P)       # [2, 128, s]
    outr = out.rearrange("(h p) s -> h p s", p=P)

    with ExitStack() as ctx:
        data = ctx.enter_context(tc.tile_pool(name="data", bufs=1))
        small = ctx.enter_context(tc.tile_pool(name="small", bufs=1))
        psum = ctx.enter_context(tc.tile_pool(name="psum", bufs=1, space="PSUM"))

        # Persistent SBUF halves of x (channel c on partition, spatial on free)
        xt0 = data.tile([P, s], F32)
        xt1 = data.tile([P, s], F32)

        # Constants. w1t packed [p, h, CS] so one DMA loads both halves;
        # b2 packed [p, h].
        w1_raw = small.tile([P, 2, CS], F32)
        w2_raw = small.tile([CS, C], F32)
        w1_sb = small.tile([P, 2, CS], F32)
        w2_sb = small.tile([CS, C], F32)
        b1_sb = small.tile([CS, 1], F32)
        b2_sb = small.tile([P, 2], F32)
        nc.gpsimd.dma_start(out=w1_raw, in_=w1t.rearrange("(h p) c -> p h c", p=P))
        nc.gpsimd.dma_start(out=w2_raw, in_=w2t)
        nc.gpsimd.dma_start(out=b1_sb, in_=b1)
        nc.gpsimd.dma_start(out=b2_sb, in_=b2.rearrange("(h p) o -> p (h o)", p=P))
        # Stage the matmul weights through DVE: PE LDWEIGHTS can encode only
        # ONE sync wait, so every matmul must depend on a single semaphore
        # (DVE's) — never on a DMA-lane sem + DVE at once.
        nc.vector.tensor_copy(w1_sb, w1_raw)
        nc.vector.tensor_copy(w2_sb, w2_raw)

        # Three independent DMA paths: SP HWDGE ring, ACT HWDGE ring, GPSIMD
        # SWDGE ring. Round-robin the big transfers across them.
        rings = [nc.sync, nc.scalar, nc.gpsimd]

        # Phase A: load x + pool. Emit ALL load triggers before the first
        # ACT activation so the ACT table load doesn't delay the ACT-ring
        # DMAs. part*[p, j] hold per-chunk partial sums.
        nld = len(load_w)
        part0 = small.tile([P, nld], F32)
        offs = []
        o = 0
        for w in load_w:
            offs.append(o)
            o += w
        # half0 on the SP ring, half1 on the ACT ring — two HWDGE rings
        # sustain ~430-460 GB/s combined for the loads (adding the SWDGE ring
        # to the load path measurably SLOWS it). The ACT ring starts ~3us
        # later (table-load + trigger issue), so the tail chunks of BOTH
        # halves go on the SP ring to make the rings finish together.
        tail_on_sync = set()
        acc = 0
        for j in range(nld - 1, -1, -1):
            if acc + load_w[j] <= s // 8:
                tail_on_sync.add(j)
                acc += load_w[j]
        for j, w in enumerate(load_w):
            sl = slice(offs[j], offs[j] + w)
            nc.sync.dma_start(out=xt0[:, sl], in_=xr[0, :, sl])
            ring1 = nc.sync if j in tail_on_sync else nc.scalar
            ring1.dma_start(out=xt1[:, sl], in_=xr[1, :, sl])

        # Warm the ACT sigmoid table set while the DMAs stream (must be ACT's
        # first ACTIVATE so Sigmoid/Copy share one table-set load).
        warm = small.tile([1, 1], F32)
        nc.vector.memset(warm, 0.0)
        nc.scalar.activation(out=warm, in_=warm, func=AF.Sigmoid)

        # half0 on DVE: read-only reduce. (In-place tensor_scalar with
        # accum_out is 2x faster on DVE but its SBUF write traffic
        # throttles the concurrent load DMAs ~430 -> ~270 GB/s.)
        part1 = small.tile([P, nld], F32)
        scr_pool = ctx.enter_context(tc.tile_pool(name="scratch", bufs=2))
        for j, w in enumerate(load_w):
            sl = slice(offs[j], offs[j] + w)
            nc.vector.reduce_sum(
                out=part0[:, j : j + 1], in_=xt0[:, sl], axis=AX.X
            )
            # half1 on ACT: Copy to scratch with accumulate.
            scr = scr_pool.tile([P, max(load_w)], F32, tag="scr")
            nc.scalar.activation(
                out=scr[:, :w],
                in_=xt1[:, sl],
                func=AF.Copy,
                bias=0.0,
                scale=1.0,
                accum_out=part1[:, j : j + 1],
            )

        # Gate: combine partials, two tiny GEMVs on PE, LeakyReLU, Sigmoid.
        sums0 = small.tile([P, 1], F32)
        sums1 = small.tile([P, 1], F32)
        nc.vector.reduce_sum(out=sums0, in_=part0, axis=AX.X)
        nc.vector.reduce_sum(out=sums1, in_=part1, axis=AX.X)

        py1 = psum.tile([CS, 1], F32)
        nc.tensor.matmul(py1, w1_sb[:, 0, :], sums0, start=True, stop=False)
        nc.tensor.matmul(py1, w1_sb[:, 1, :], sums1, start=False, stop=True)

        # t = py1/s + b1 ; y1 = max(t, 0.2*t)  (== leaky_relu(t))
        t = small.tile([CS, 1], F32)
        u = small.tile([CS, 1], F32)
        y1_sb = small.tile([CS, 1], F32)
        nc.vector.tensor_scalar(t, py1, 1.0 / s, b1_sb, ALU.mult, ALU.add)
        nc.vector.tensor_scalar_mul(u, t, NEG_SLOPE)
        nc.vector.tensor_max(y1_sb, t, u)

        py20 = psum.tile([P, 1], F32)
        py21 = psum.tile([P, 1], F32)
        nc.tensor.matmul(py20, w2_sb[:, 0:P], y1_sb, start=True, stop=True)
        nc.tensor.matmul(py21, w2_sb[:, P : 2 * P], y1_sb, start=True, stop=True)

        y2_sb = small.tile([P, 2], F32)
        nc.scalar.activation(
            out=y2_sb[:, 0:1], in_=py20, func=AF.Sigmoid, bias=b2_sb[:, 0:1], scale=1.0
        )
        nc.scalar.activation(
            out=y2_sb[:, 1:2], in_=py21, func=AF.Sigmoid, bias=b2_sb[:, 1:2], scale=1.0
        )

        # Phase B: scale x by the gate in place and store, chunked so DMA-out
        # overlaps the multiplies. DVE takes half0, ACT takes half1; stores
        # round-robin over the three DMA rings.
        ring_i = 0
        o = 0
        for w in scale_w:
            sl = slice(o, o + w)
            o += w
            nc.vector.tensor_scalar_mul(
                out=xt0[:, sl], in0=xt0[:, sl], scalar1=y2_sb[:, 0:1]
            )
            nc.scalar.activation(
                out=xt1[:, sl], in_=xt1[:, sl], func=AF.Copy, bias=0.0,
                scale=y2_sb[:, 1:2],
            )
            rings[ring_i % 3].dma_start(out=outr[0, :, sl], in_=xt0[:, sl])
            ring_i += 1
            rings[ring_i % 3].dma_start(out=outr[1, :, sl], in_=xt1[:, sl])
            ring_i += 1


def build_calayer_bass(s=S, trn_type="TRN2"):
    # Bacc (not raw Bass): its compile() pipeline splits multi-wait sync_info
    # into event semaphores — TRN2 instructions encode at most one wait.
    nc = bacc.Bacc(trn_type=trn_type)
    x = nc.dram_tensor("x", [C, s], F32, kind="ExternalInput")
    w1t = nc.dram_tensor("w1t", [C, CS], F32, kind="ExternalInput")
    b1 = nc.dram_tensor("b1", [CS, 1], F32, kind="ExternalInput")
    w2t = nc.dram_tensor("w2t", [CS, C], F32, kind="ExternalInput")
    b2 = nc.dram_tensor("b2", [C, 1], F32, kind="ExternalInput")
    out = nc.dram_tensor("out", [C, s], F32, kind="ExternalOutput")
    with tile.TileContext(nc) as tc:
        _body(
            tc, x[:, :], w1t[:, :], b1[:, :], w2t[:, :], b2[:, :], out[:, :], s,
        )
    nc.finalize()  # Bacc.finalize runs compile(): wait-splitting, reg alloc
    return nc


_NC_CACHE = None
RUN_KWARGS = {}      # test harness may inject trace=True etc.
LAST_RESULT = None   # BassKernelResults of the most recent run


def _get_nc():
    global _NC_CACHE
    if _NC_CACHE is None:
        _NC_CACHE = build_calayer_bass()
    return _NC_CACHE


def kernel(x, w1, b1, w2, b2):
    global LAST_RESULT
    x = np.asarray(x, dtype=np.float32)
    xf = np.ascontiguousarray(x.reshape(B, C, S))
    w1t_h = np.ascontiguousarray(np.asarray(w1, dtype=np.float32).T)  # [C, CS]
    w2t_h = np.ascontiguousarray(np.asarray(w2, dtype=np.float32).T)  # [CS, C]
    b1_h = np.ascontiguousarray(np.asarray(b1, dtype=np.float32).reshape(CS, 1))
    b2_h = np.ascontiguousarray(np.asarray(b2, dtype=np.float32).reshape(C, 1))

    in_maps = [
        {"x": xf[b], "w1t": w1t_h, "b1": b1_h, "w2t": w2t_h, "b2": b2_h}
        for b in range(B)
    ]
    res = run_bass_kernel_spmd(
        _get_nc(), in_maps, core_ids=list(range(N_CORES)), **RUN_KWARGS
    )
    LAST_RESULT = res
    out = np.stack([res.results[b]["out"] for b in range(B)], axis=0)
    return out.reshape(B, C, H, W)


# revision 30
# speedup vs baseline: 1.0900x; 1.0006x over previous
"""Trainium2 Bass kernel for CALayer (squeeze-excitation channel attention).

Reference computation (per batch sample b):
    y  = mean(x[b], spatial)              # [C]
    y1 = leaky_relu(w1 @ y + b1, 0.2)     # [16]
    y2 = sigmoid(w2 @ y1 + b2)            # [C]
    out[b] = x[b] * y2[:, None, None]

Sharding: data-parallel over batch — 8 samples, 8 NeuronCores, one sample per
core, weights replicated, no cross-core communication.

Per-core plan (memory-bound, x[b] = 16 MiB fits in SBUF):
  - x[b] viewed as [256, 16384] lives in SBUF as two [128, 16384] channel
    halves; loaded in column chunks (half0 via the SP HWDGE ring, half1 via
    the ACT HWDGE ring) so pooling overlaps the loads.
  - Pooling: DVE reduce_sum per chunk for half0; ACT Copy-with-accum_out for
    half1 (both engines in parallel, hidden under DMA).
  - Gate: PE matmuls against pre-transposed weights (transposed on host),
    LeakyReLU decomposed as max(t, 0.2*t) on DVE, Sigmoid+bias on ACT
    (table set pre-warmed at kernel start so the ~2.7us load hides under DMA).
  - Scale: per-partition broadcast multiply by the gate — DVE tensor_scalar
    for half0, ACT Copy-with-scale for half1, in place, chunked; stores
    issued per chunk on the GPSIMD SWDGE ring (engine otherwise idle).

HBM traffic per core: 16 MiB in + 16 MiB out (the roofline for this op).
"""

from contextlib import ExitStack

import numpy as np

import concourse.bacc as bacc
import concourse.bass as bass
import concourse.mybir as mybir
import concourse.tile as tile
from concourse.bass_utils import run_bass_kernel_spmd

F32 = mybir.dt.float32
AF = mybir.ActivationFunctionType
ALU = mybir.AluOpType
AX = mybir.AxisListType

B, C, H, W = 8, 256, 128, 128
S = H * W          # 16384 spatial elements
CS = 16            # squeezed channels
NEG_SLOPE = 0.2
N_CORES = 8
P = 128            # SBUF partitions


def _plans(s):
    """(load_widths, scale_widths) per half. Loads taper down so the last
    pooled chunk is small (shrinks the gate bubble); scales taper up so the
    first store fires quickly after the gate."""
    if s == 16384:
        lw = [4096, 4096, 4096, 2048, 1024, 1024]
        return lw, lw[::-1]
    n = max(1, s // 512)
    return [s // n] * n, [s // n] * n


def _body(tc, x, w1t, b1, w2t, b2, out, s, load_w=None, scale_w=None):
    """Emit the per-core kernel. APs: x/out [C, s], w1t [C, CS], b1 [CS, 1],
    w2t [CS, C], b2 [C, 1]."""
    nc = tc.nc
    if load_w is None:
        load_w, scale_w = _plans(s)
    assert sum(load_w) == s and sum(scale_w) == s
    xr = x.rearrange("(h p) s -> h p s", p=P)       # [2, 128, s]
    outr = out.rearrange("(h p) s -> h p s", p=P)

    with ExitStack() as ctx:
        data = ctx.enter_context(tc.tile_pool(name="data", bufs=1))
        small = ctx.enter_context(tc.tile_pool(name="small", bufs=1))
        psum = ctx.enter_context(tc.tile_pool(name="psum", bufs=1, space="PSUM"))

        # Persistent SBUF halves of x (channel c on partition, spatial on free)
        xt0 = data.tile([P, s], F32)
        xt1 = data.tile([P, s], F32)

        # Constants. w1t packed [p, h, CS] so one DMA loads both halves;
        # b2 packed [p, h].
        w1_raw = small.tile([P, 2, CS], F32)
        w2_raw = small.tile([CS, C], F32)
        w1_sb = small.tile([P, 2, CS], F32)
        w2_sb = small.tile([CS, C], F32)
        b1_sb = small.tile([CS, 1], F32)
        b2_sb = small.tile([P, 2], F32)
        nc.gpsimd.dma_start(out=w1_raw, in_=w1t.rearrange("(h p) c -> p h c", p=P))
        nc.gpsimd.dma_start(out=w2_raw, in_=w2t)
        nc.gpsimd.dma_start(out=b1_sb, in_=b1)
        nc.gpsimd.dma_start(out=b2_sb, in_=b2.rearrange("(h p) o -> p (h o)", p=P))
        # Stage the matmul weights through DVE: PE LDWEIGHTS can encode only
        # ONE sync wait, so every matmul must depend on a single semaphore
        # (DVE's) — never on a DMA-lane sem + DVE at once.
        nc.vector.tensor_copy(w1_sb, w1_raw)
        nc.vector.tensor_copy(w2_sb, w2_raw)

        # Three independent DMA paths: SP HWDGE ring, ACT HWDGE ring, GPSIMD
        # SWDGE ring. Round-robin the big transfers across them.
        rings = [nc.sync, nc.scalar, nc.gpsimd]

        # Phase A: load x + pool. Emit ALL load triggers before the first
        # ACT activation so the ACT table load doesn't delay the ACT-ring
        # DMAs. part*[p, j] hold per-chunk partial sums.
        nld = len(load_w)
        part0 = small.tile([P, nld], F32)
        offs = []
        o = 0
        for w in load_w:
            offs.append(o)
            o += w
        # half0 on the SP ring, half1 on the ACT ring — two HWDGE rings
        # sustain ~430-460 GB/s combined for the loads (adding the SWDGE ring
        # to the load path measurably SLOWS it). The ACT ring starts ~3us
        # later (table-load + trigger issue), so the tail chunks of BOTH
        # halves go on the SP ring to make the rings finish together.
        tail_on_sync = set()
        acc = 0
        for j in range(nld - 1, -1, -1):
            if acc + load_w[j] <= s // 8:
                tail_on_sync.add(j)
                acc += load_w[j]
        for j, w in enumerate(load_w):
            sl = slice(offs[j], offs[j] + w)
            nc.sync.dma_start(out=xt0[:, sl], in_=xr[0, :, sl])
            ring1 = nc.sync if j in tail_on_sync else nc.scalar
            ring1.dma_start(out=xt1[:, sl], in_=xr[1, :, sl])

        # Warm the ACT sigmoid table set while the DMAs stream (must be ACT's
        # first ACTIVATE so Sigmoid/Copy share one table-set load).
        warm = small.tile([1, 1], F32)
        nc.vector.memset(warm, 0.0)
        nc.scalar.activation(out=warm, in_=warm, func=AF.Sigmoid)

        # half0 on DVE: read-only reduce. (In-place tensor_scalar with
        # accum_out is 2x faster on DVE but its SBUF write traffic
        # throttles the concurrent load DMAs ~430 -> ~270 GB/s.)
        part1 = small.tile([P, nld], F32)
        scr_pool = ctx.enter_context(tc.tile_pool(name="scratch", bufs=2))
        for j, w in enumerate(load_w):
            sl = slice(offs[j], offs[j] + w)
            nc.vector.reduce_sum(
                out=part0[:, j : j + 1], in_=xt0[:, sl], axis=AX.X
            )
            # half1 on ACT: Copy to scratch with accumulate.
            scr = scr_pool.tile([P, max(load_w)], F32, tag="scr")
            nc.scalar.activation(
                out=scr[:, :w],
                in_=xt1[:, sl],
                func=AF.Copy,
                bias=0.0,
                scale=1.0,
                accum_out=part1[:, j : j + 1],
            )

        # Gate: combine partials, two tiny GEMVs on PE, LeakyReLU, Sigmoid.
        sums0 = small.tile([P, 1], F32)
        sums1 = small.tile([P, 1], F32)
        nc.vector.reduce_sum(out=sums0, in_=part0, axis=AX.X)
        nc.vector.reduce_sum(out=sums1, in_=part1, axis=AX.X)

        py1 = psum.tile([CS, 1], F32)
        nc.tensor.matmul(py1, w1_sb[:, 0, :], sums0, start=True, stop=False)
        nc.tensor.matmul(py1, w1_sb[:, 1, :], sums1, start=False, stop=True)

        # t = py1/s + b1 ; y1 = max(t, 0.2*t)  (== leaky_relu(t))
        t = small.tile([CS, 1], F32)
        u = small.tile([CS, 1], F32)
        y1_sb = small.tile([CS, 1], F32)
        nc.vector.tensor_scalar(t, py1, 1.0 / s, b1_sb, ALU.mult, ALU.add)
        nc.vector.tensor_scalar_mul(u, t, NEG_SLOPE)
        nc.vector.tensor_max(y1_sb, t, u)

        py20 = psum.tile([P, 1], F32)
        py21 = psum.tile([P, 1], F32)
        nc.tensor.matmul(py20, w2_sb[:, 0:P], y1_sb, start=True, stop=True)
        nc.tensor.matmul(py21, w2_sb[:, P : 2 * P], y1_sb, start=True, stop=True)

        y2_sb = small.tile([P, 2], F32)
        nc.scalar.activation(
            out=y2_sb[:, 0:1], in_=py20, func=AF.Sigmoid, bias=b2_sb[:, 0:1], scale=1.0
        )
        nc.scalar.activation(
            out=y2_sb[:, 1:2], in_=py21, func=AF.Sigmoid, bias=b2_sb[:, 1:2], scale=1.0
        )

        # Phase B: scale x by the gate in place and store, chunked so DMA-out
        # overlaps the multiplies. DVE takes half0, ACT takes half1; stores
        # on the two HWDGE rings (half0 -> SP, half1 -> ACT).
        o = 0
        for w in scale_w:
            sl = slice(o, o + w)
            o += w
            nc.vector.tensor_scalar_mul(
                out=xt0[:, sl], in0=xt0[:, sl], scalar1=y2_sb[:, 0:1]
            )
            nc.scalar.activation(
                out=xt1[:, sl], in_=xt1[:, sl], func=AF.Copy, bias=0.0,
                scale=y2_sb[:, 1:2],
            )
            nc.sync.dma_start(out=outr[0, :, sl], in_=xt0[:, sl])
            nc.scalar.dma_start(out=outr[1, :, sl], in_=xt1[:, sl])


def build_calayer_bass(s=S, trn_type="TRN2"):
    # Bacc (not raw Bass): its compile() pipeline splits multi-wait sync_info
    # into event semaphores — TRN2 instructions encode at most one wait.
    nc = bacc.Bacc(trn_type=trn_type)
    x = nc.dram_tensor("x", [C, s], F32, kind="ExternalInput")
    w1t = nc.dram_tensor("w1t", [C, CS], F32, kind="ExternalInput")
    b1 = nc.dram_tensor("b1", [CS, 1], F32, kind="ExternalInput")
    w2t = nc.dram_tensor("w2t", [CS, C], F32, kind="ExternalInput")
    b2 = nc.dram_tensor("b2", [C, 1], F32, kind="ExternalInput")
    out = nc.dram_tensor("out", [C, s], F32, kind="ExternalOutput")
    with tile.TileContext(nc) as tc:
        _body(
            tc, x[:, :], w1t[:, :], b1[:, :], w2t[:, :], b2[:, :], out[:, :], s,
        )
    nc.finalize()  # Bacc.finalize runs compile(): wait-splitting, reg alloc
    return nc


_NC_CACHE = None
RUN_KWARGS = {}      # test harness may inject trace=True etc.
LAST_RESULT = None   # BassKernelResults of the most recent run


def _get_nc():
    global _NC_CACHE
    if _NC_CACHE is None:
        _NC_CACHE = build_calayer_bass()
    return _NC_CACHE


def kernel(x, w1, b1, w2, b2):
    global LAST_RESULT
    x = np.asarray(x, dtype=np.float32)
    xf = np.ascontiguousarray(x.reshape(B, C, S))
    w1t_h = np.ascontiguousarray(np.asarray(w1, dtype=np.float32).T)  # [C, CS]
    w2t_h = np.ascontiguousarray(np.asarray(w2, dtype=np.float32).T)  # [CS, C]
    b1_h = np.ascontiguousarray(np.asarray(b1, dtype=np.float32).reshape(CS, 1))
    b2_h = np.ascontiguousarray(np.asarray(b2, dtype=np.float32).reshape(C, 1))

    in_maps = [
        {"x": xf[b], "w1t": w1t_h, "b1": b1_h, "w2t": w2t_h, "b2": b2_h}
        for b in range(B)
    ]
    res = run_bass_kernel_spmd(
        _get_nc(), in_maps, core_ids=list(range(N_CORES)), **RUN_KWARGS
    )
    LAST_RESULT = res
    out = np.stack([res.results[b]["out"] for b in range(B)], axis=0)
    return out.reshape(B, C, H, W)
